# revision 3
# baseline (speedup 1.0000x reference)
"""LoCon1d (position-specific conv1d) Trainium2 kernel, v2.

out[b,o,s] = sum_{c,k} xpad[b,c,s+k] * w[o,c,s,k] + bias[o,s]
shapes: x (16,64,1024) f32, w (64,64,1024,3) f32, bias (64,1024) f32.

Sharding: sequence-parallel over 8 cores, 128 positions each.

Per-core mapping (shifted-stationary, column-tiled):
  Positions split into half-blocks (j, 64+j), j in 0..63, packed
  block-diagonally into the 128-partition contraction dim:
  partitions 0:64 = Cin for position j, 64:128 = Cin for position
  64+j; batch columns 0:16 <-> j, 16:32 <-> 64+j (zeros baked in on
  host).

  Groups of 4 consecutive pairs a in 0..3 (j = 4G+a). Per group and
  tap kk, FOUR column-tiled matmuls run concurrently (tile_position
  (0, 32a)): stationary = x window [128, 32] for pair a, moving =
  w[G, kk, pair a] as [128, 64] fp8. All land in one psum [128, 64]:
  row 32a+m = (pair a, batch-ext m), every slot valid (no diagonal
  waste). Taps accumulate (start=kk==0, stop=kk==2).

  All 3 weight taps ride fp8 (e4m3): rel-err ~1.6e-2 (< 2e-2 gate),
  weight bytes 1.5 MiB/core. x rides f16 in 3 overlapping slabs; w in
  5 need-ordered chunks across the sync/scalar HWDGE queues + gpsimd
  SWDGE; one [128,64] f32->f16 vector copy per group; out ships in 3
  full-partition contiguous DMAs. Dummy matmuls warm the PE HAM clock
  during the DMA lead-in. Bias is added during host assembly.
"""

import os
os.environ.setdefault("NEURON_SCRATCHPAD_PAGE_SIZE", "2048")

import numpy as np

import concourse.bass as bass
import concourse.mybir as mybir
import concourse.tile as tile
from concourse import bacc, bass_utils

N_CORES = 8
B, CIN, COUT, S, K = 16, 64, 64, 1024, 3
SC = S // N_CORES          # positions per core (128)
H = SC // 2                # half-block (64)
NG = H // 4                # matmul groups (16), 4 pairs each
TW = H + K - 1             # x window length per half-block (66)
# x slabs: (t0, t1, Gfirst, Glast+1); group G reads x cols [4G, 4G+7)
XSPLITS = [(0, 15, 0, 3), (12, 40, 3, 9), (36, 66, 9, 16)]
# w chunks: (g0, g1)
WCHUNKS = [(0, 1), (1, 3), (3, 6), (6, 8), (8, 10), (10, 12), (12, 14), (14, 16)]
# out chunks: (g0, g1)
OCHUNKS = [(0, 8), (8, 14), (14, 16)]
N_DUMMY = 4                # PE warm-up matmuls during DMA lead-in

_DT = {"f32": mybir.dt.float32, "bf16": mybir.dt.bfloat16,
       "f16": mybir.dt.float16}

DTYPE = "f16"


def _np_dt(dt):
    if dt == "bf16":
        import ml_dtypes
        return ml_dtypes.bfloat16
    if dt == "f16":
        return np.float16
    return np.float32


def build_bass(dtype=DTYPE):
    dt = _DT[dtype]
    nc = bacc.Bacc("TRN2", target_bir_lowering=False, debug=False,
                   num_devices=N_CORES)
    dt8 = mybir.dt.float8e4
    xc = nc.dram_tensor("xc", [128, TW, 16], dt, kind="ExternalInput")
    # per (p, G): 3 taps x 256 fp8 bytes = 768 B, viewed as 384 f16.
    # fp8 byte 256*kk + 64*a + o  <->  w[tap kk, pair a, cout o]
    wr = nc.dram_tensor("wr", [128, NG, 384], dt, kind="ExternalInput")
    # out[32a+m, G, o]: pair j=4G+a; m<16: batch m pos j; else 64+j
    out = nc.dram_tensor("out", [128, NG, 64], dt, kind="ExternalOutput")

    with tile.TileContext(nc) as tc:
        with (
            tc.tile_pool(name="dpool", bufs=1) as dpool,
            tc.tile_pool(name="xpool", bufs=len(XSPLITS)) as xpool,
            tc.tile_pool(name="wpool", bufs=len(WCHUNKS)) as wpool,
            tc.tile_pool(name="opool", bufs=len(OCHUNKS)) as opool,
            tc.tile_pool(name="psum", bufs=8, space="PSUM") as pspool,
        ):
            # PE warm-up: a few long matmuls on a memset tile so the HAM
            # clock gate opens during the DMA lead-in.
            dm = dpool.tile([128, 640], dt, tag="dm")
            nc.vector.memset(dm[:, :], 0)
            ps_tiles = []
            for i in range(N_DUMMY):
                ps = pspool.tile([128, 512], mybir.dt.float32, tag="ps")
                ps_tiles.append(ps)
                nc.tensor.matmul(ps[:, :], lhsT=dm[:, 0:128],
                                 rhs=dm[:, 128:640], start=True, stop=True)

            # DMA issues, need-ordered across the three queues.
            x_tiles = []
            xc_tiles = []
            for si, (t0, t1, g0, g1) in enumerate(XSPLITS):
                xt = xpool.tile([128, t1 - t0, 32], dt, tag=f"xt{si}",
                                name=f"xt{si}")
                x_tiles.append(xt)
                xct = xpool.tile([128, t1 - t0, 16], dt, tag=f"xct{si}",
                                 name=f"xct{si}")
                xc_tiles.append(xct)
            # zeros form the block-diagonal complement; expansion copies
            # overlay the compact data after its DMA lands
            for si in range(3):
                nc.vector.memset(x_tiles[si][:, :, :], 0)
            w_tiles = []
            for wi, (g0, g1) in enumerate(WCHUNKS):
                wt = wpool.tile([128, g1 - g0, 384], dt, tag=f"wt{wi}",
                                name=f"wt{wi}")
                w_tiles.append(wt)

            def xdma(eng, si):
                eng.dma_start(out=xc_tiles[si][:, :, :],
                              in_=xc.ap()[:, XSPLITS[si][0]:XSPLITS[si][1], :])

            def wdma(eng, wi):
                eng.dma_start(out=w_tiles[wi][:, :, :],
                              in_=wr.ap()[:, WCHUNKS[wi][0]:WCHUNKS[wi][1], :])

            def xexpand(eng, si):
                cp = eng.copy if eng is nc.scalar else eng.tensor_copy
                cp(out=x_tiles[si][0:64, :, 0:16],
                   in_=xc_tiles[si][0:64, :, :])
                cp(out=x_tiles[si][64:128, :, 16:32],
                   in_=xc_tiles[si][64:128, :, :])

            xdma(nc.sync, 0)
            wdma(nc.scalar, 0)    # [0:1]
            wdma(nc.sync, 1)      # [1:3]
            xdma(nc.gpsimd, 1)
            wdma(nc.scalar, 2)    # [3:6]
            xdma(nc.gpsimd, 2)
            wdma(nc.sync, 3)      # [6:8]
            wdma(nc.scalar, 4)    # [8:10]
            wdma(nc.sync, 5)      # [10:12]
            wdma(nc.scalar, 6)    # [12:14]
            wdma(nc.sync, 7)      # [14:16]
            xexpand(nc.vector, 0)
            xexpand(nc.vector, 1)
            nc.vector.tensor_copy(out=x_tiles[2][0:64, :, 0:16],
                                  in_=xc_tiles[2][0:64, :, :])
            nc.scalar.copy(out=x_tiles[2][64:128, :, 16:32],
                           in_=xc_tiles[2][64:128, :, :])

            o_tiles = []
            for oi, (g0, g1) in enumerate(OCHUNKS):
                ot = opool.tile([128, g1 - g0, 64], dt, tag=f"ot{oi}",
                                name=f"ot{oi}")
                o_tiles.append(ot)

            oi = 0
            for G in range(NG):
                for wi, (g0, g1) in enumerate(WCHUNKS):
                    if g0 <= G < g1:
                        wt = w_tiles[wi]
                        gl = G - g0
                        break
                for si, (t0s, t1s, gf, glast) in enumerate(XSPLITS):
                    if gf <= G < glast:
                        lhs_t = x_tiles[si]
                        t0 = 4 * G - t0s
                        break
                ps = pspool.tile([128, 512], mybir.dt.float32, tag="ps")
                for kk in range(K):
                    for a in range(4):
                        rhs = wt[:, gl, 128 * kk + 32 * a:
                                 128 * kk + 32 * a + 32].bitcast(dt8)
                        nc.tensor.matmul(
                            ps[32 * a:32 * a + 32, 0:64],
                            lhsT=lhs_t[:, t0 + kk + a, :],
                            rhs=rhs,
                            start=(kk == 0),
                            stop=(kk == K - 1),
                            tile_position=(0, 32 * a),
                        )
                og0, og1 = OCHUNKS[oi]
                ot = o_tiles[oi]
                if G % 2 == 0:
                    nc.vector.tensor_copy(out=ot[:, G - og0, :],
                                          in_=ps[:, 0:64])
                else:
                    nc.scalar.copy(out=ot[:, G - og0, :], in_=ps[:, 0:64])
                if G == og1 - 1:
                    eng = (nc.scalar, nc.sync, nc.scalar)[oi]
                    eng.dma_start(out=out.ap()[:, og0:og1, :],
                                  in_=ot[:, :, :])
                    oi += 1
    nc.compile()
    return nc


def prep_inputs(input, weight, bias, dtype=DTYPE):
    """Host-side shard + relayout. Returns list of per-core input dicts."""
    import ml_dtypes
    npdt = _np_dt(dtype)
    xpad = np.pad(np.asarray(input, np.float32), ((0, 0), (0, 0), (1, 1)))
    w = np.asarray(weight, np.float32)
    in_maps = []
    for i in range(N_CORES):
        s0 = i * SC
        # x: [p, t, b_ext] block-diagonal
        xa = xpad[:, :, s0:s0 + TW]             # (B, CIN, TW)
        xb = xpad[:, :, s0 + H:s0 + H + TW]
        xcc = np.empty((128, TW, 16), np.float32)
        xcc[:64] = xa.transpose(1, 2, 0)
        xcc[64:] = xb.transpose(1, 2, 0)
        # w: [p, G, kk, a, o] ; pair j = 4G + a
        ws = w[:, :, s0:s0 + SC, :]             # (COUT, CIN, SC, K)
        wa = ws[:, :, :H, :].reshape(COUT, CIN, NG, 4, K)
        wb = ws[:, :, H:, :].reshape(COUT, CIN, NG, 4, K)
        wrc = np.empty((128, NG, K, 4, COUT), np.float32)
        wrc[:64] = wa.transpose(1, 2, 4, 3, 0)  # (c, G, kk, a, o)
        wrc[64:] = wb.transpose(1, 2, 4, 3, 0)
        fp8 = np.ascontiguousarray(
            wrc.reshape(128, NG, K * 256).astype(ml_dtypes.float8_e4m3fn))
        blob = fp8.view(np.uint8).reshape(128, NG, 768).view(np.float16)
        in_maps.append({
            "xc": np.ascontiguousarray(xcc.astype(npdt)),
            "wr": np.ascontiguousarray(blob),
        })
    return in_maps


def assemble_output(results, bias):
    full = np.empty((B, COUT, S), np.float32)
    for i, r in enumerate(results):
        s0 = i * SC
        oc = np.asarray(r["out"], np.float32)    # (128, NG, 64)
        # oc[32a+m, G, o] -> position s0 + 4G + a (+H if m>=16), batch m%16
        blk = oc.reshape(4, 2, 16, NG, COUT)     # (a, half, m, G, o)
        # full[b, o, s0 + half*H + 4G + a] = blk[a, half, b, G, o]
        v = blk.transpose(2, 4, 1, 3, 0)         # (m, o, half, G, a)
        full[:, :, s0:s0 + SC] = v.reshape(B, COUT, SC)
    full += np.asarray(bias, np.float32)[None, :, :]
    return full


_CACHED = {}


def run(inputs, dtype=DTYPE, trace=False):
    if dtype not in _CACHED:
        _CACHED[dtype] = build_bass(dtype)
    nc = _CACHED[dtype]
    in_maps = prep_inputs(inputs["input"], inputs["weight"], inputs["bias"],
                          dtype)
    res = bass_utils.run_bass_kernel_spmd(
        nc, in_maps, core_ids=list(range(N_CORES)), trace=trace)
    return assemble_output(res.results, inputs["bias"]), res


def kernel(input, weight, bias):
    out, _ = run({"input": input, "weight": weight, "bias": bias},
                 trace=False)
    return out


# revision 4
# speedup vs baseline: 1.0822x; 1.0822x over previous
"""LoCon1d (position-specific conv1d) Trainium2 kernel, v2.

out[b,o,s] = sum_{c,k} xpad[b,c,s+k] * w[o,c,s,k] + bias[o,s]
shapes: x (16,64,1024) f32, w (64,64,1024,3) f32, bias (64,1024) f32.

Sharding: sequence-parallel over 8 cores, 128 positions each.

Per-core mapping (shifted-stationary, column-tiled):
  Positions split into half-blocks (j, 64+j), j in 0..63, packed
  block-diagonally into the 128-partition contraction dim:
  partitions 0:64 = Cin for position j, 64:128 = Cin for position
  64+j; batch columns 0:16 <-> j, 16:32 <-> 64+j (zeros baked in on
  host).

  Groups of 4 consecutive pairs a in 0..3 (j = 4G+a). Per group and
  tap kk, FOUR column-tiled matmuls run concurrently (tile_position
  (0, 32a)): stationary = x window [128, 32] for pair a, moving =
  w[G, kk, pair a] as [128, 64] fp8. All land in one psum [128, 64]:
  row 32a+m = (pair a, batch-ext m), every slot valid (no diagonal
  waste). Taps accumulate (start=kk==0, stop=kk==2).

  All 3 weight taps ride fp8 (e4m3): rel-err ~1.6e-2 (< 2e-2 gate),
  weight bytes 1.5 MiB/core. x rides f16 in 3 overlapping slabs; w in
  5 need-ordered chunks across the sync/scalar HWDGE queues + gpsimd
  SWDGE; one [128,64] f32->f16 vector copy per group; out ships in 3
  full-partition contiguous DMAs. Dummy matmuls warm the PE HAM clock
  during the DMA lead-in. Bias is added during host assembly.
"""

import os
os.environ.setdefault("NEURON_SCRATCHPAD_PAGE_SIZE", "2048")

import numpy as np

import concourse.bass as bass
import concourse.mybir as mybir
import concourse.tile as tile
from concourse import bacc, bass_utils

N_CORES = 8
B, CIN, COUT, S, K = 16, 64, 64, 1024, 3
SC = S // N_CORES          # positions per core (128)
H = SC // 2                # half-block (64)
NG = H // 4                # matmul groups (16), 4 pairs each
TW = H + K - 1             # x window length per half-block (66)
# x slabs: (t0, t1, Gfirst, Glast+1); group G reads x cols [4G, 4G+7)
XSPLITS = [(0, 15, 0, 3), (12, 40, 3, 9), (36, 66, 9, 16)]
# w chunks: (g0, g1)
WCHUNKS = [(0, 1), (1, 3), (3, 6), (6, 8), (8, 10), (10, 12), (12, 14), (14, 16)]
# out chunks: (g0, g1)
OCHUNKS = [(0, 8), (8, 14), (14, 16)]
N_DUMMY = 4                # PE warm-up matmuls during DMA lead-in

_DT = {"f32": mybir.dt.float32, "bf16": mybir.dt.bfloat16,
       "f16": mybir.dt.float16}

DTYPE = "f16"


def _np_dt(dt):
    if dt == "bf16":
        import ml_dtypes
        return ml_dtypes.bfloat16
    if dt == "f16":
        return np.float16
    return np.float32


def build_bass(dtype=DTYPE):
    dt = _DT[dtype]
    nc = bacc.Bacc("TRN2", target_bir_lowering=False, debug=False,
                   num_devices=N_CORES)
    dt8 = mybir.dt.float8e4
    xc = nc.dram_tensor("xc", [128, TW, 16], dt, kind="ExternalInput")
    # per (p, G): 3 taps x 256 fp8 bytes = 768 B, viewed as 384 f16.
    # fp8 byte 256*kk + 64*a + o  <->  w[tap kk, pair a, cout o]
    wr = nc.dram_tensor("wr", [128, NG, 384], dt, kind="ExternalInput")
    # out[32a+m, G, o]: pair j=4G+a; m<16: batch m pos j; else 64+j
    out = nc.dram_tensor("out", [128, NG, 64], dt, kind="ExternalOutput")

    with tile.TileContext(nc) as tc:
        with (
            tc.tile_pool(name="dpool", bufs=1) as dpool,
            tc.tile_pool(name="xpool", bufs=len(XSPLITS)) as xpool,
            tc.tile_pool(name="wpool", bufs=len(WCHUNKS)) as wpool,
            tc.tile_pool(name="opool", bufs=len(OCHUNKS)) as opool,
            tc.tile_pool(name="psum", bufs=8, space="PSUM") as pspool,
        ):
            # PE warm-up: a few long matmuls on a memset tile so the HAM
            # clock gate opens during the DMA lead-in.
            dm = dpool.tile([128, 640], dt, tag="dm")
            nc.vector.memset(dm[:, :], 0)
            ps_tiles = []
            for i in range(N_DUMMY):
                ps = pspool.tile([128, 512], mybir.dt.float32, tag="ps")
                ps_tiles.append(ps)
                nc.tensor.matmul(ps[:, :], lhsT=dm[:, 0:128],
                                 rhs=dm[:, 128:640], start=True, stop=True)

            # DMA issues, need-ordered across the three queues.
            x_tiles = []
            xc_tiles = []
            for si, (t0, t1, g0, g1) in enumerate(XSPLITS):
                xt = xpool.tile([128, t1 - t0, 32], dt, tag=f"xt{si}",
                                name=f"xt{si}")
                x_tiles.append(xt)
                xct = xpool.tile([128, t1 - t0, 16], dt, tag=f"xct{si}",
                                 name=f"xct{si}")
                xc_tiles.append(xct)
            # zeros form the block-diagonal complement; expansion copies
            # overlay the compact data after its DMA lands
            for si in range(3):
                nc.vector.memset(x_tiles[si][:, :, :], 0)
            w_tiles = []
            for wi, (g0, g1) in enumerate(WCHUNKS):
                wt = wpool.tile([128, g1 - g0, 384], dt, tag=f"wt{wi}",
                                name=f"wt{wi}")
                w_tiles.append(wt)

            def xdma(eng, si):
                eng.dma_start(out=xc_tiles[si][:, :, :],
                              in_=xc.ap()[:, XSPLITS[si][0]:XSPLITS[si][1], :])

            def wdma(eng, wi):
                eng.dma_start(out=w_tiles[wi][:, :, :],
                              in_=wr.ap()[:, WCHUNKS[wi][0]:WCHUNKS[wi][1], :])

            def xexpand(eng, si):
                cp = eng.copy if eng is nc.scalar else eng.tensor_copy
                cp(out=x_tiles[si][0:64, :, 0:16],
                   in_=xc_tiles[si][0:64, :, :])
                cp(out=x_tiles[si][64:128, :, 16:32],
                   in_=xc_tiles[si][64:128, :, :])

            xdma(nc.sync, 0)
            wdma(nc.scalar, 0)    # [0:1]
            wdma(nc.sync, 1)      # [1:3]
            xdma(nc.gpsimd, 1)
            wdma(nc.scalar, 2)    # [3:6]
            xdma(nc.gpsimd, 2)
            wdma(nc.sync, 3)      # [6:8]
            wdma(nc.scalar, 4)    # [8:10]
            wdma(nc.sync, 5)      # [10:12]
            wdma(nc.scalar, 6)    # [12:14]
            wdma(nc.sync, 7)      # [14:16]
            xexpand(nc.vector, 0)
            xexpand(nc.vector, 1)
            nc.vector.tensor_copy(out=x_tiles[2][0:64, :, 0:16],
                                  in_=xc_tiles[2][0:64, :, :])
            nc.scalar.copy(out=x_tiles[2][64:128, :, 16:32],
                           in_=xc_tiles[2][64:128, :, :])

            o_tiles = []
            for oi, (g0, g1) in enumerate(OCHUNKS):
                ot = opool.tile([128, g1 - g0, 64], dt, tag=f"ot{oi}",
                                name=f"ot{oi}")
                o_tiles.append(ot)

            oi = 0
            for G in range(NG):
                for wi, (g0, g1) in enumerate(WCHUNKS):
                    if g0 <= G < g1:
                        wt = w_tiles[wi]
                        gl = G - g0
                        break
                for si, (t0s, t1s, gf, glast) in enumerate(XSPLITS):
                    if gf <= G < glast:
                        lhs_t = x_tiles[si]
                        t0 = 4 * G - t0s
                        break
                ps = pspool.tile([128, 512], mybir.dt.float32, tag="ps")
                for kk in range(K):
                    for a in range(4):
                        rhs = wt[:, gl, 128 * kk + 32 * a:
                                 128 * kk + 32 * a + 32].bitcast(dt8)
                        nc.tensor.matmul(
                            ps[32 * a:32 * a + 32, 0:64],
                            lhsT=lhs_t[:, t0 + kk + a, :],
                            rhs=rhs,
                            start=(kk == 0),
                            stop=(kk == K - 1),
                            tile_position=(0, 32 * a),
                        )
                og0, og1 = OCHUNKS[oi]
                ot = o_tiles[oi]
                if G % 2 == 1:
                    nc.vector.tensor_copy(out=ot[:, G - og0, :],
                                          in_=ps[:, 0:64])
                else:
                    nc.scalar.copy(out=ot[:, G - og0, :], in_=ps[:, 0:64])
                if G == og1 - 1:
                    eng = (nc.scalar, nc.scalar, nc.sync)[oi]
                    eng.dma_start(out=out.ap()[:, og0:og1, :],
                                  in_=ot[:, :, :])
                    oi += 1
    nc.compile()
    return nc


def prep_inputs(input, weight, bias, dtype=DTYPE):
    """Host-side shard + relayout. Returns list of per-core input dicts."""
    import ml_dtypes
    npdt = _np_dt(dtype)
    xpad = np.pad(np.asarray(input, np.float32), ((0, 0), (0, 0), (1, 1)))
    w = np.asarray(weight, np.float32)
    in_maps = []
    for i in range(N_CORES):
        s0 = i * SC
        # x: [p, t, b_ext] block-diagonal
        xa = xpad[:, :, s0:s0 + TW]             # (B, CIN, TW)
        xb = xpad[:, :, s0 + H:s0 + H + TW]
        xcc = np.empty((128, TW, 16), np.float32)
        xcc[:64] = xa.transpose(1, 2, 0)
        xcc[64:] = xb.transpose(1, 2, 0)
        # w: [p, G, kk, a, o] ; pair j = 4G + a
        ws = w[:, :, s0:s0 + SC, :]             # (COUT, CIN, SC, K)
        wa = ws[:, :, :H, :].reshape(COUT, CIN, NG, 4, K)
        wb = ws[:, :, H:, :].reshape(COUT, CIN, NG, 4, K)
        wrc = np.empty((128, NG, K, 4, COUT), np.float32)
        wrc[:64] = wa.transpose(1, 2, 4, 3, 0)  # (c, G, kk, a, o)
        wrc[64:] = wb.transpose(1, 2, 4, 3, 0)
        fp8 = np.ascontiguousarray(
            wrc.reshape(128, NG, K * 256).astype(ml_dtypes.float8_e4m3fn))
        blob = fp8.view(np.uint8).reshape(128, NG, 768).view(np.float16)
        in_maps.append({
            "xc": np.ascontiguousarray(xcc.astype(npdt)),
            "wr": np.ascontiguousarray(blob),
        })
    return in_maps


def assemble_output(results, bias):
    full = np.empty((B, COUT, S), np.float32)
    for i, r in enumerate(results):
        s0 = i * SC
        oc = np.asarray(r["out"], np.float32)    # (128, NG, 64)
        # oc[32a+m, G, o] -> position s0 + 4G + a (+H if m>=16), batch m%16
        blk = oc.reshape(4, 2, 16, NG, COUT)     # (a, half, m, G, o)
        # full[b, o, s0 + half*H + 4G + a] = blk[a, half, b, G, o]
        v = blk.transpose(2, 4, 1, 3, 0)         # (m, o, half, G, a)
        full[:, :, s0:s0 + SC] = v.reshape(B, COUT, SC)
    full += np.asarray(bias, np.float32)[None, :, :]
    return full


_CACHED = {}


def run(inputs, dtype=DTYPE, trace=False):
    if dtype not in _CACHED:
        _CACHED[dtype] = build_bass(dtype)
    nc = _CACHED[dtype]
    in_maps = prep_inputs(inputs["input"], inputs["weight"], inputs["bias"],
                          dtype)
    res = bass_utils.run_bass_kernel_spmd(
        nc, in_maps, core_ids=list(range(N_CORES)), trace=trace)
    return assemble_output(res.results, inputs["bias"]), res


def kernel(input, weight, bias):
    out, _ = run({"input": input, "weight": weight, "bias": bias},
                 trace=False)
    return out
